# revision 25
# baseline (speedup 1.0000x reference)
import sys

sys.path.insert(0, "/opt/trn_rl_repo")

import numpy as np

H = 1024
NH = 16
HD = 64
L = 2048
B = 2
N_CORES = 8
HEADS_PER_CORE = 4
KT = H // 128
IC = L // 512
JT = L // 128

_CACHE = {}


def _build():
    import concourse.bass as bass
    import concourse.mybir as mybir
    import concourse.tile as tile
    from concourse import bacc

    F32 = mybir.dt.float32
    F32R = mybir.dt.float32r
    F16 = mybir.dt.float16
    EXP = mybir.ActivationFunctionType.Exp

    nc = bacc.Bacc("TRN2", target_bir_lowering=False, debug=False,
                   num_devices=N_CORES)

    xT_d = nc.declare_dram_parameter("xT", [H, L], F16, isOutput=False)
    wqT_d = nc.declare_dram_parameter("wqT", [H, 256], F16, isOutput=False)
    wkT_d = nc.declare_dram_parameter("wkT", [H, 256], F16, isOutput=False)
    wvT_d = nc.declare_dram_parameter("wvT", [H, 256], F16, isOutput=False)
    woutT_d = nc.declare_dram_parameter("woutT", [256, H], F16, isOutput=False)
    bq_d = nc.declare_dram_parameter("bq", [256, 1], F32, isOutput=False)
    bk_d = nc.declare_dram_parameter("bk", [256, 1], F32, isOutput=False)
    bvrep_d = nc.declare_dram_parameter("bvrep", [128, 256], F32, isOutput=False)
    ident_d = nc.declare_dram_parameter("ident", [128, 128], F16, isOutput=False)
    out_d = nc.declare_dram_parameter("out", [L, H], F16, isOutput=True)

    with tile.TileContext(nc) as tc, nc.allow_low_precision(
            reason="fp16 operand tiles; all reductions accumulate in fp32 "
                   "PSUM"):
        with tc.tile_pool(name="sbW", bufs=1) as sbW, \
             tc.tile_pool(name="sbA", bufs=1) as sbA, \
             tc.tile_pool(name="sbPT", bufs=4) as sbPT, \
             tc.tile_pool(name="sbZ", bufs=2) as sbZ, \
             tc.tile_pool(name="sbOut", bufs=4) as sbOut, \
             tc.tile_pool(name="psST", bufs=2, space="PSUM") as psST, \
             tc.tile_pool(name="psACC", bufs=1, space="PSUM") as psACC, \
             tc.tile_pool(name="psCH", bufs=2, space="PSUM") as psCH:

            xT_r = xT_d.rearrange("(k p) n -> p k n", p=128)
            wq_r = wqT_d.rearrange("(k p) d -> p k d", p=128)
            wk_r = wkT_d.rearrange("(k p) d -> p k d", p=128)
            wq = sbW.tile([128, KT, 256], F16, tag="wq", name="wq")
            wk = sbW.tile([128, KT, 256], F16, tag="wk", name="wk")
            x0 = sbW.tile([128, KT, 512], F16, tag="x0", name="x0")
            for half in range(2):
                k0 = 4 * half
                nc.sync.dma_start(out=x0[:, k0:k0 + 4, :], in_=xT_r[:, k0:k0 + 4, 0:512])
                nc.sync.dma_start(out=wq[:, k0:k0 + 4, :], in_=wq_r[:, k0:k0 + 4, :])
                nc.sync.dma_start(out=wk[:, k0:k0 + 4, :], in_=wk_r[:, k0:k0 + 4, :])
            xt = [x0]
            bq_sb = sbW.tile([128, 2], F32)
            nc.sync.dma_start(out=bq_sb, in_=bq_d.rearrange("(q p) c -> p (q c)", p=128))
            bk_sb = sbW.tile([128, 2], F32)
            nc.sync.dma_start(out=bk_sb, in_=bk_d.rearrange("(q p) c -> p (q c)", p=128))
            wv = sbW.tile([128, KT, 256], F16)
            nc.sync.dma_start(out=wv, in_=wvT_d.rearrange("(k p) d -> p k d", p=128))
            for c in range(1, IC):
                xc = sbW.tile([128, KT, 512], F16, tag=f"x{c}", name=f"x{c}")
                nc.sync.dma_start(out=xc, in_=xT_r[:, :, 512 * c:512 * (c + 1)])
                xt.append(xc)
                if c == 1:
                    bvrep = sbW.tile([128, 256], F32)
                    nc.sync.dma_start(out=bvrep, in_=bvrep_d[:, :])
            wout = sbW.tile([128, 2, H], F16)
            nc.sync.dma_start(out=wout, in_=woutT_d.rearrange("(q p) e -> p q e", p=128))
            ident = sbW.tile([128, 128], F16)
            nc.sync.dma_start(out=ident, in_=ident_d[:, :])

            qTc = [[sbA.tile([128, 512], F32R, tag=f"qT{p}_{i}", name=f"qT{p}_{i}")
                    for i in range(IC)] for p in range(2)]
            kTc = [[sbA.tile([128, 512], F32R, tag=f"kT{p}_{i}", name=f"kT{p}_{i}")
                    for i in range(IC)] for p in range(2)]
            vtj = [sbA.tile([128, HEADS_PER_CORE, 65], F16, tag=f"vt{jt}",
                            name=f"vt{jt}") for jt in range(JT)]
            for jt in range(JT):
                nc.gpsimd.memset(vtj[jt][:, :, 64:65], 1.0)
            o2T = [[sbA.tile([128, 512], F16, tag=f"o2T{p}_{ic}",
                             name=f"o2T{p}_{ic}")
                    for ic in range(IC)] for p in range(2)]

            def q_mm(p, ic, ps, k):
                nc.tensor.matmul(ps, lhsT=wq[:, k, 128 * p:128 * p + 128],
                                 rhs=xt[ic][:, k, :],
                                 start=(k == 0), stop=(k == KT - 1))

            def k_mm(p, c, ps, k):
                nc.tensor.matmul(ps, lhsT=wk[:, k, 128 * p:128 * p + 128],
                                 rhs=xt[c][:, k, :],
                                 start=(k == 0), stop=(k == KT - 1))

            def q_fin(p, ic, ps):
                nc.vector.tensor_scalar_add(qTc[p][ic], ps, bq_sb[:, p:p + 1])

            def k_fin(p, c, ps):
                nc.vector.tensor_scalar_add(kTc[p][c], ps, bk_sb[:, p:p + 1])

            def chain_parts(kind, p, i, nparts=4):
                state = {}
                per = KT // nparts
                mm = q_mm if kind == "q" else k_mm
                fin = q_fin if kind == "q" else k_fin

                def part(j):
                    def f():
                        if j == 0:
                            state["ps"] = psCH.tile([128, 512], F32, tag="ch",
                                                    name=f"ps_{kind}")
                        for k in range(per * j, per * (j + 1)):
                            mm(p, i, state["ps"], k)
                        if j == nparts - 1:
                            fin(p, i, state["ps"])
                    return f
                return [part(j) for j in range(nparts)]

            def whole_chain(kind, p, i):
                for f in chain_parts(kind, p, i, nparts=1):
                    f()

            def v_chain(jt):
                c, jl = jt // 4, jt % 4
                ps = psCH.tile([128, 256], F32, tag="ch", name="ps_v")
                for k in range(KT):
                    nc.tensor.matmul(ps,
                                     lhsT=xt[c][:, k, 128 * jl:128 * jl + 128],
                                     rhs=wv[:, k, :],
                                     start=(k == 0), stop=(k == KT - 1))
                nc.vector.tensor_add(
                    vtj[jt][:, :, 0:64],
                    ps.rearrange("p (h d) -> p h d", h=HEADS_PER_CORE),
                    bvrep.rearrange("p (h d) -> p h d", h=HEADS_PER_CORE))

            def s_pair(p, ic, jt):
                c, jl = jt // 4, jt % 4
                st = psST.tile([128, 1024], F32, tag="st", name="st")
                nc.tensor.matmul(st[:, 0:512],
                                 lhsT=kTc[p][c][0:64, 128 * jl:128 * jl + 128],
                                 rhs=qTc[p][ic][0:64, :],
                                 start=True, stop=True)
                nc.tensor.matmul(st[:, 512:1024],
                                 lhsT=kTc[p][c][64:128, 128 * jl:128 * jl + 128],
                                 rhs=qTc[p][ic][64:128, :],
                                 start=True, stop=True)
                return st

            def pv_step(p, jt, pt, accs):
                for t in range(4):
                    for h in range(2):
                        o0 = 130 * (t % 2) + 65 * h
                        nc.tensor.matmul(
                            accs[t // 2][:, o0:o0 + 65],
                            lhsT=pt[:, 512 * h + 128 * t:512 * h + 128 * t + 128],
                            rhs=vtj[jt][:, 2 * p + h, :],
                            start=(jt == 0 and t % 2 == 0 and h == 0),
                            stop=(jt == JT - 1 and t % 2 == 1 and h == 1),
                            skip_group_check=True)

            def attention_body(p, ic, fillers=None, st_in=None, nxt=None):
                accA = psACC.tile([128, 260], F32, tag="accA", name="accA")
                accB = psACC.tile([128, 260], F32, tag="accB", name="accB")
                accs = [accA, accB]

                st = st_in if st_in is not None else s_pair(p, ic, 0)
                st_out = None
                pts = [None, None]
                for jt in range(JT):
                    if fillers:
                        for f in fillers.get(jt, ()):
                            f()
                    if jt + 1 < JT:
                        st_next = s_pair(p, ic, jt + 1)
                    elif nxt is not None:
                        st_next = st_out = s_pair(nxt[0], nxt[1], 0)
                    else:
                        st_next = None
                    pt = sbPT.tile([128, 1024], F16, tag="pt", name="pt")
                    nc.scalar.activation(pt, st, EXP, scale=float(HD) ** -0.5)
                    pts[jt % 2] = pt
                    if jt >= 1:
                        pv_step(p, jt - 1, pts[(jt - 1) % 2], accs)
                    st = st_next
                pv_step(p, JT - 1, pts[(JT - 1) % 2], accs)
                return (p, ic, accs), st_out

            def norm_mul(o2sb, accs, a, zrec):
                out_v = o2sb[:, 2 * a:2 * a + 2, :].rearrange(
                    "p t (h c) -> p (t h) c", c=64)
                acc_v = accs[a].rearrange("p (r c) -> p r c", c=65)[:, :, 0:64]
                z_v = zrec[:, 4 * a:4 * a + 4].rearrange(
                    "p (r one) -> p r one", one=1)
                acc_b, z_b = bass.broadcast_tensor_aps(acc_v, z_v)
                nc.vector.tensor_mul(out_v, acc_b, z_b)

            def normalize_fin(pend):
                p, ic, accs = pend
                zrec = sbZ.tile([128, 8], F32, tag="zrec", name="zrec")
                for a in range(2):
                    nc.vector.reciprocal(
                        zrec[:, 4 * a:4 * a + 4],
                        accs[a].rearrange("p (r c) -> p r c", c=65)[:, :, 64])
                o2sb = sbZ.tile([128, 4, 128], F16, tag="o2sb", name="o2sb")
                for a in range(2):
                    norm_mul(o2sb, accs, a, zrec)
                nc.sync.dma_start_transpose(
                    out=o2T[p][ic].rearrange("p (t q) -> p t q", t=4),
                    in_=o2sb.rearrange("p t q -> p (t q)"))

            def proj_unit(ic, itl, ec, ost_box, tail=False):
                t0 = 512 * ic + 128 * itl
                pool, tg = (psST, "st") if (tail and ec == 1) else (psCH, "ch")
                ps = pool.tile([128, 512], F32, tag=tg, name="ps_o")
                e0 = 512 * ec
                nc.tensor.matmul(ps,
                                 lhsT=o2T[0][ic][:, 128 * itl:128 * itl + 128],
                                 rhs=wout[:, 0, e0:e0 + 512],
                                 start=True, stop=False)
                nc.tensor.matmul(ps,
                                 lhsT=o2T[1][ic][:, 128 * itl:128 * itl + 128],
                                 rhs=wout[:, 1, e0:e0 + 512],
                                 start=False, stop=True)
                if ec == 0:
                    ost_box["t"] = sbOut.tile([128, 1024], F16, tag="ost",
                                              name="ost")
                ost = ost_box["t"]
                if tail and ec == 0:
                    nc.scalar.copy(ost[:, e0:e0 + 512], ps)
                else:
                    nc.vector.tensor_copy(ost[:, e0:e0 + 512], ps)
                if ec == 1:
                    nc.sync.dma_start(out=out_d[t0:t0 + 128, :], in_=ost)

            def proj_units(ic, itl, tail=False):
                box = {}
                return [
                    (lambda i, t, e, b: lambda: proj_unit(i, t, e, b, tail))(
                        ic, itl, ec, box)
                    for ec in range(2)
                ]

            wuseed = sbZ.tile([128, 2], F16, tag="wus", name="wus")
            nc.vector.memset(wuseed[:, :], 0.0)
            wups = psCH.tile([128, 16], F32, tag="ch", name="wu")
            for _ in range(880):
                nc.tensor.matmul(wups[0:2, 0:2], lhsT=wuseed[:, 0:2],
                                 rhs=wuseed[:, 0:2], start=True, stop=True,
                                 skip_group_check=True)
            ps_q0 = psCH.tile([128, 512], F32, tag="ch", name="ps_q0")
            ps_k0 = psCH.tile([128, 512], F32, tag="ch", name="ps_k0")
            for k in range(KT):
                q_mm(0, 0, ps_q0, k)
                k_mm(0, 0, ps_k0, k)
            q_fin(0, 0, ps_q0)
            k_fin(0, 0, ps_k0)

            segs = []
            f00 = {jt: [] for jt in range(JT)}
            for jt in range(JT):
                f00[jt].append((lambda j: lambda: v_chain(j))(jt))
            for ci in range(1, IC):
                for j, f in enumerate(chain_parts("k", 0, ci, nparts=2)):
                    f00[4 * ci - 4 + 1 + j].append(f)
            for j, f in enumerate(chain_parts("q", 0, 1, nparts=2)):
                f00[13 + j].append(f)
            segs.append((0, 0, f00))

            for ic in range(1, IC):
                fq = {}
                for j, f in enumerate(chain_parts("q", 1, ic - 1)):
                    fq.setdefault(1 + j, []).append(f)
                for j, f in enumerate(chain_parts("k", 1, ic - 1)):
                    fq.setdefault(5 + j, []).append(f)
                if ic < IC - 1:
                    for j, f in enumerate(chain_parts("q", 0, ic + 1)):
                        fq.setdefault(9 + j, []).append(f)
                segs.append((0, ic, fq))
            for ic in range(IC):
                fp = {}
                if ic == 0:
                    for j, f in enumerate(chain_parts("q", 1, IC - 1)):
                        fp.setdefault(2 + j, []).append(f)
                    for j, f in enumerate(chain_parts("k", 1, IC - 1)):
                        fp.setdefault(7 + j, []).append(f)
                else:
                    units = [u for itl in range(4) for u in proj_units(ic - 1, itl)]
                    for u, f in zip((4, 5, 6, 7, 8, 9, 11, 12), units):
                        fp.setdefault(u, []).append(f)
                segs.append((1, ic, fp))

            st_hand = None
            pend = None
            for idx, (p, ic, fillers) in enumerate(segs):
                nxt = segs[idx + 1][:2] if idx + 1 < len(segs) else None
                pend, st_hand = attention_body(p, ic, fillers,
                                               st_in=st_hand, nxt=nxt)
                if idx + 1 < len(segs):
                    normalize_fin(pend)
            p, ic, accs = pend
            zrec = sbZ.tile([128, 8], F32, tag="zrec", name="zrec")
            for a in range(2):
                nc.vector.reciprocal(
                    zrec[:, 4 * a:4 * a + 4],
                    accs[a].rearrange("p (r c) -> p r c", c=65)[:, :, 64])
            o2sb = sbZ.tile([128, 4, 128], F16, tag="o2sb", name="o2sb")
            for t in range(4):
                out_v = o2sb[:, t:t + 1, :].rearrange("p t (h c) -> p (t h) c", c=64)
                a = t // 2
                acc_v = accs[a].rearrange("p (r c) -> p r c", c=65)[
                    :, 2 * (t % 2):2 * (t % 2) + 2, 0:64]
                z_v = zrec[:, 4 * a + 2 * (t % 2):4 * a + 2 * (t % 2) + 2].rearrange(
                    "p (r one) -> p r one", one=1)
                acc_b, z_b = bass.broadcast_tensor_aps(acc_v, z_v)
                nc.vector.tensor_mul(out_v, acc_b, z_b)
                pst = psACC.tile([128, 128], F16, tag="accA" if t % 2 == 0 else "accB",
                                 name="pst")
                nc.tensor.transpose(pst, o2sb[:, t, :], ident)
                nc.vector.tensor_copy(o2T[p][ic][:, 128 * t:128 * (t + 1)], pst)
                for f in proj_units(IC - 1, t, tail=True):
                    f()
    nc.compile()
    return nc


def _get_nc():
    if "nc" not in _CACHE:
        _CACHE["nc"] = _build()
    return _CACHE["nc"]


def make_in_maps(x, w_qkv, b_qkv, w_out):
    f32 = np.float32
    x = np.asarray(x, f32)
    w_qkv = np.asarray(w_qkv, f32)
    b_qkv = np.asarray(b_qkv, f32)
    w_out = np.asarray(w_out, f32)
    in_maps = []
    for c in range(N_CORES):
        b = c // 4
        g = c % 4
        r0, r1 = 64 * 4 * g, 64 * 4 * (g + 1)
        in_maps.append({
            "xT": np.ascontiguousarray(x[b].T).astype(np.float16),
            "wqT": np.ascontiguousarray(w_qkv[r0:r1, :].T).astype(np.float16),
            "wkT": np.ascontiguousarray(w_qkv[H + r0:H + r1, :].T).astype(np.float16),
            "wvT": np.ascontiguousarray(w_qkv[2 * H + r0:2 * H + r1, :].T).astype(np.float16),
            "woutT": np.ascontiguousarray(w_out[:, r0:r1].T).astype(np.float16),
            "bq": b_qkv[r0:r1].reshape(256, 1).copy(),
            "bk": b_qkv[H + r0:H + r1].reshape(256, 1).copy(),
            "bvrep": np.tile(b_qkv[2 * H + r0:2 * H + r1][None, :], (128, 1)),
            "ident": np.eye(128, dtype=np.float16),
        })
    return in_maps


def assemble(results, b_out):
    out = np.empty((B, L, H), np.float32)
    for b in range(B):
        acc = results[4 * b]["out"].astype(np.float32)
        for c in range(4 * b + 1, 4 * b + 4):
            acc = acc + results[c]["out"]
        out[b] = acc + np.asarray(b_out, np.float32)[None, :]
    return out


def kernel(x, w_qkv, b_qkv, w_out, b_out):
    from concourse.bass_utils import run_bass_kernel_spmd

    nc = _get_nc()
    in_maps = make_in_maps(x, w_qkv, b_qkv, w_out)
    res = run_bass_kernel_spmd(nc, in_maps, core_ids=list(range(N_CORES)))
    return assemble(res.results, b_out)


if __name__ == "__main__":
    rng = np.random.default_rng(0)
    x = rng.standard_normal((B, L, H), dtype=np.float32)
    w_qkv = (rng.standard_normal((3 * H, H), dtype=np.float32) / np.sqrt(H)).astype(np.float32)
    b_qkv = (rng.standard_normal(3 * H).astype(np.float32) * 0.01)
    w_out = (rng.standard_normal((H, H), dtype=np.float32) / np.sqrt(H)).astype(np.float32)
    b_out = (rng.standard_normal(H).astype(np.float32) * 0.01)
    out = kernel(x, w_qkv, b_qkv, w_out, b_out)
    print("kernel output", out.shape, out.dtype)
